# revision 39
# baseline (speedup 1.0000x reference)
"""L1-distance attention kernel for Trainium2 (8 NeuronCores, SPMD).

Problem: q, k: [B=2, T=512, H=8, D=64] fp32
         out[b,s,t,h] = -sum_d |q[b,s,h,d] - k[b,t,h,d]| / sqrt(D)

Sharding: 16 (b,h) pairs across 8 cores, 2 pairs per core, stacked in the
SBUF partition dim (pair0 -> partitions 0:64 holding d, pair1 -> 64:128).

Math (on bf16-rounded inputs; the only error vs fp32 is input rounding):
  |q-k| = (q+k) - 2*min(q,k)  and  |q-k| = 2*relu(q-k) - (q-k), so
  -scale*sum_d|q-k| = 2*scale*sum_d min(k,q_s) - scale*K_t - scale*Q_s   (DVE)
                    = -2*scale*sum_d relu(q_s-k) - scale*K_t + scale*Q_s (ACT)
where Q_s = sum_d q[d,s], K_t = sum_d k[d,t].

Design notes (cost-model driven; TimelineSim 118.1us vs 488us for the
fp32 accumulate-everything baseline; steady-state 109.0us per 512-query
pass = the PE-stream floor, ~100% PE efficiency; the remaining 9us are
the input-DMA chain ~3.7us and the output-drain chain ~4.2us, both
dominated by fixed per-DMA dispatch/semaphore constants):
  - All matmul operands bf16 (1 cycle/row vs 4 for fp32, so each [128,512]
    reduction matmul is ~213ns hot); min(bf16,bf16) is exact and
    +-0.25/-0.125 selector weights are exact in bf16.  fp8+DoubleRow would
    halve the PE floor but fails the 2e-2 rel-err gate (verified on host).
  - Producers: DVE tensor_scalar_min (bf16 4x mode, 194ns) on strips
    0/1/3, ScalarE Relu (612ns) on strip 2 -> every jj round is 3 DVE
    (582ns) + 1 ACT (669ns), both under the 852ns PE round.
  - Per-query selector weights are 32-col sliding windows over two tiny
    [128, 62] buffers (sign per producer type): no per-query weight DMA.
  - Inputs ship as one bf16 pack (k|q|zdve|zact, first) + wk + qs packs;
    fp32 q for the scalar/bias APs is derived on-device (bf16->f32 copy).
  - K_t correction: ONE matmul total -> SBUF, folded with the qs bias into
    per-group kcorqs tiles (DVE, spread out); output path is a DVE
    tensor_tensor add (PSUM + kcorqs) so the PE does only ad-matmuls.
  - Warmup matmuls (into group 6's bank, overwritten later) run during
    the input DMA to hold the PE p-state/HAM ramp; a dummy activation at
    t~1.4us pulls the ~1.3us ACT function-table load into the DMA window
    so the first real Relu producer isn't stalled behind it.
  - bf16 output staging (<=2^-9 extra rounding, ~10% of the error budget)
    halves output DMA volume; the host upcasts while unscrambling.
  - c-strips interleave inner-most so consecutive matmuls target
    different PE column groups (real-HW col-tiling concurrency).
  - Last group's output tensor_tensor/DMA is column-split to shorten the
    final drain chain.
"""

import os

import numpy as np
import ml_dtypes

os.environ.setdefault("MYCRO_LOCAL_CACHE", "1")

B, T, H, D = 2, 512, 8, 64
NCORES = 8
NGROUPS = 8  # query groups of 64 -> one PSUM tile each
SCALE = 1.0 / float(np.sqrt(np.float64(D)))  # 0.125
NWARM = 2

BF16 = ml_dtypes.bfloat16

# bf16 pack 1 (producer-critical, first DMA): k | q | zdve | zact
KOFF = 0
QOFF = T  # 512
ZDVEOFF = 2 * T  # 1024
ZACTOFF = ZDVEOFF + 62  # 1086
BPACK = ZACTOFF + 62  # 1148
# bf16 pack 2 (second DMA): wk [128, 128]


# Producer assignment per (c, jj) slot: DVE bf16 tensor_scalar_min vs
# ScalarE Relu.  Real-HW measured rates with rotated destinations
# (hwbench.py): DVE ~274ns, ACT ~700-777ns, PE matmul ~211ns -> 128/512 ACT
# keeps real producers (~105us) just under the real PE wall (~108us), and
# every jj round is exactly 3 DVE + 1 ACT in the cost model too.
def slot_is_act(c, jj):
    return c == 2


_cached = {}


def _build_module(reps=1):
    from concourse import bacc, tile
    import concourse.mybir as mybir

    f32 = mybir.dt.float32
    bf16 = mybir.dt.bfloat16
    nc = bacc.Bacc(
        "TRN2",
        target_bir_lowering=False,
        debug=False,
        enable_asserts=False,
        num_devices=1,
    )
    # fpack: the per-(row,group) output bias qs (fp32).  q ships as bf16 in
    # bpack; the fp32 copy the scalar/bias APs need is derived on-device.
    fpack_dram = nc.dram_tensor("fpack", [128, NGROUPS], f32, kind="ExternalInput")
    bpack_dram = nc.dram_tensor("bpack", [128, BPACK], bf16, kind="ExternalInput")
    wk_dram = nc.dram_tensor("wkp", [128, 128], bf16, kind="ExternalInput")
    # bf16 output staging: rounds the final sums (<=2^-9 rel, ~10% of the
    # error budget) and halves the output DMA volume incl. the tail-critical
    # last transfer; the host upcasts to fp32 while unscrambling.
    out_dram = nc.dram_tensor("out", [NGROUPS, 128, T], bf16, kind="ExternalOutput")

    with tile.TileContext(nc) as tc:
        with (
            tc.tile_pool(name="const", bufs=1) as cpool,
            tc.tile_pool(name="ad", bufs=10) as adpool,
            tc.tile_pool(name="osb", bufs=3) as opool,
            tc.tile_pool(name="psum", bufs=1, space="PSUM") as ppool,
        ):
            warm_sb = cpool.tile([1, T], bf16, tag="warm")
            fpack_sb = cpool.tile([128, NGROUPS], f32, tag="fpack")
            bpack_sb = cpool.tile([128, BPACK], bf16, tag="bpack")
            wk_sb = cpool.tile([128, 128], bf16, tag="wk")
            q32_sb = cpool.tile([128, T], f32, tag="q32")
            actwarm_sb = cpool.tile([1, 16], bf16, tag="actwarm")
            nc.vector.memset(warm_sb[:], 1.0)
            # Dummy activation right after the memset: pulls the ~1.3us ACT
            # function-table load into the input-DMA window instead of
            # stalling the first real Relu producer.
            nc.scalar.activation(
                actwarm_sb[:],
                warm_sb[0:1, 0:16],
                mybir.ActivationFunctionType.Relu,
                scale=-1.0,
            )
            nc.sync.dma_start(bpack_sb[:], bpack_dram[:])
            nc.sync.dma_start(wk_sb[:], wk_dram[:])
            nc.sync.dma_start(fpack_sb[:], fpack_dram[:])

            qs_ap = fpack_sb[:, 0:NGROUPS]
            k_ap = bpack_sb[:, KOFF : KOFF + T]
            wk_ap = wk_sb[:]
            # fp32 q for the DVE scalar / ACT bias APs (values bf16-exact)
            nc.vector.tensor_copy(q32_sb[:], bpack_sb[:, QOFF : QOFF + T])
            q_ap = q32_sb[:]

            def wsel(c, jj):
                off = (ZACTOFF if slot_is_act(c, jj) else ZDVEOFF) + 30 - 2 * jj
                return bpack_sb[:, off : off + 32]

            # All 8 PSUM banks, one per group, allocated up front (no reuse).
            ptiles = [
                ppool.tile([128, T], f32, name=f"acc{g}", tag=f"acc{g}")
                for g in range(NGROUPS)
            ]

            # PE warmup: keeps the p-state ramp going through the input-DMA
            # wait and the early producer-limited fill gaps, so real matmuls
            # run at full clock.  Targets group 6's bank, whose real strips
            # overwrite it (start=True) much later.  (Bank 7 holds the kcor
            # result until its SBUF copy.)
            def warm():
                nc.tensor.matmul(
                    ptiles[NGROUPS - 2][0:16, :],
                    warm_sb[0:1, 0:16],
                    warm_sb[0:1, :],
                    start=True,
                    stop=True,
                    skip_group_check=True,
                )

            for _ in range(NWARM):
                warm()

            def producer(c, jj, g):
                s = 64 * g + 16 * c + jj
                ad = adpool.tile([128, T], bf16, tag="ad")
                if slot_is_act(c, jj):
                    nc.scalar.activation(
                        ad[:],
                        k_ap,
                        mybir.ActivationFunctionType.Relu,
                        bias=q_ap[:, s : s + 1],
                        scale=-1.0,
                    )
                else:
                    nc.vector.tensor_scalar_min(ad[:], k_ap, q_ap[:, s : s + 1])
                return ad

            def admm(psum_t, ad, c, jj):
                # Strip-wise accumulation: each matmul contributes its 2 rows
                # (other 30 rows get zeros from the zero weight columns), so
                # the strip must accumulate across jj: start only on jj==0.
                nc.tensor.matmul(
                    psum_t[32 * c : 32 * c + 32, :],
                    wsel(c, jj),
                    ad[:],
                    start=(jj == 0),
                    stop=(jj == 15),
                    tile_position=(0, 32 * c),
                    skip_group_check=True,
                )

            # K_t correction computed once: one matmul into group 7's PSUM
            # bank (not written by real strips until ~100us in), copied to
            # SBUF mid-stream (DVE has ~270ns slack per round), then folded
            # per group with the qs bias into kcorqs tiles that the output
            # add (DVE tensor_tensor) applies.  Keeping these preps out of
            # the DVE FIFO's head lets the first producers start right when
            # the input DMA lands instead of behind the prep chain.
            nc.tensor.matmul(
                ptiles[NGROUPS - 1][:, :],
                wk_ap,
                k_ap,
                start=True,
                stop=True,
                tile_position=(0, 0),
                skip_group_check=True,
            )
            kcor_sb = cpool.tile([128, T], f32, tag="kcor")
            kcq = [
                cpool.tile([128, T], f32, name=f"kcq{g}", tag=f"kcq{g}")
                for g in range(NGROUPS)
            ]
            kcq_done = set()

            def copy_kcor():
                nc.vector.tensor_copy(kcor_sb[:], ptiles[NGROUPS - 1][:, :])

            def prep_kcq(g):
                if g < NGROUPS and g not in kcq_done:
                    kcq_done.add(g)
                    nc.vector.tensor_scalar_add(
                        kcq[g][:], kcor_sb[:], qs_ap[:, g : g + 1]
                    )

            def outcopy(psum_t, g, split=False):
                ob = opool.tile([128, T], bf16, tag="ob")
                if not split:
                    nc.vector.tensor_tensor(
                        ob[:], psum_t[:, :], kcq[g][:], mybir.AluOpType.add
                    )
                    nc.sync.dma_start(out_dram[g], ob[:])
                else:
                    # last group: column-split so the final DMA chain starts
                    # after half a tensor_tensor instead of a full one
                    h = T // 2
                    for i in range(2):
                        cs = slice(i * h, (i + 1) * h)
                        nc.vector.tensor_tensor(
                            ob[:, cs], psum_t[:, cs], kcq[g][:, cs],
                            mybir.AluOpType.add,
                        )
                        nc.sync.dma_start(out_dram[g][:, cs], ob[:, cs])

            for g in range(NGROUPS * reps):
                g = g % NGROUPS
                psum_t = ptiles[g]
                # c-strips interleaved inner-most: consecutive matmuls
                # hit different PE column groups and overlap on HW.
                for jj in range(16):
                    for c in range(4):
                        admm(psum_t, producer(c, jj, g), c, jj)
                    if g == 0 and jj == 2:
                        copy_kcor()
                    if g == 0 and jj == 5:
                        prep_kcq(0)
                    if jj == 8:
                        prep_kcq(g + 1)
                outcopy(psum_t, g, split=(g == NGROUPS - 1))

    nc.compile()
    return nc


def _host_weights():
    """zdve/zact: [128, 62] bf16 sliding-window selector buffers; window
    [30-2jj : 62-2jj] has +-2*scale at (rows 0:64, col 2jj) and
    (rows 64:128, col 2jj+1).  wk: [128, 128] -scale K-sum selector."""
    zdve = np.zeros((128, 62), np.float32)
    zdve[0:64, 30] = 2.0 * SCALE
    zdve[64:128, 31] = 2.0 * SCALE
    zact = np.zeros((128, 62), np.float32)
    zact[0:64, 30] = -2.0 * SCALE
    zact[64:128, 31] = -2.0 * SCALE
    wk = np.zeros((128, 128), np.float32)
    wk[0:64, 0::2] = -SCALE
    wk[64:128, 1::2] = -SCALE
    return zdve.astype(BF16), zact.astype(BF16), wk.astype(BF16)


def _host_qsum(qc):
    """qc: [128, T] per-core stacked q^T (bf16-rounded values). Returns qs
    [128, NGROUPS] fp32: row r = 32c + 2jj + p of group g gets
    -+scale*sum_d q[pair p, d, s] with s = 64g + 16c + jj
    (sign + for ACT slots, - for DVE slots)."""
    qsum = qc.astype(np.float64).reshape(2, 64, T).sum(axis=1)  # [pair, s]
    qs = np.empty((128, NGROUPS), np.float64)
    for g in range(NGROUPS):
        for c in range(4):
            for jj in range(16):
                s = 64 * g + 16 * c + jj
                sign = 1.0 if slot_is_act(c, jj) else -1.0
                for p in range(2):
                    qs[32 * c + 2 * jj + p, g] = sign * SCALE * qsum[p, s]
    return qs.astype(np.float32)


def get_module(reps=1):
    key = ("nc", reps)
    nc = _cached.get(key)
    if nc is None:
        nc = _build_module(reps)
        _cached[key] = nc
    return nc


def make_in_maps(q, k):
    """Shard full [B,T,H,D] q/k into 8 per-core input maps (packed)."""
    q = np.asarray(q, dtype=np.float32)
    k = np.asarray(k, dtype=np.float32)
    # [B, T, H, D] -> [B, H, D, T] -> [B*H, D, T]
    qt = np.ascontiguousarray(q.transpose(0, 2, 3, 1)).reshape(B * H, D, T)
    kt = np.ascontiguousarray(k.transpose(0, 2, 3, 1)).reshape(B * H, D, T)
    zdve, zact, wk = _host_weights()
    in_maps = []
    for c in range(NCORES):
        qc = (
            np.ascontiguousarray(qt[2 * c : 2 * c + 2].reshape(128, T))
            .astype(BF16)
            .astype(np.float32)
        )
        kc = np.ascontiguousarray(kt[2 * c : 2 * c + 2].reshape(128, T)).astype(BF16)
        fpack = _host_qsum(qc)
        bpack = np.concatenate([kc, qc.astype(BF16), zdve, zact], axis=1)
        in_maps.append(
            {
                "fpack": fpack,
                "bpack": np.ascontiguousarray(bpack),
                "wkp": wk,
            }
        )
    return in_maps


def assemble_output(core_outs):
    """core_outs: list of 8 arrays [NGROUPS, 128, T] -> full [B, T, T, H]."""
    outf = np.empty((B, T, T, H), np.float32)
    for c in range(NCORES):
        o = np.asarray(core_outs[c]).astype(np.float32)
        o = o.reshape(NGROUPS, 4, 16, 2, T)
        # row r = 32c + 2jj + p in group g  ->  query s = 64g + 16c + jj, pair p
        o = o.transpose(3, 0, 1, 2, 4).reshape(2, T, T)
        for p in range(2):
            pg = 2 * c + p
            b, h = divmod(pg, H)
            outf[b, :, :, h] = o[p]
    return outf


def kernel(q, k):
    from concourse.bass_utils import run_bass_kernel_spmd

    nc = get_module()
    in_maps = make_in_maps(q, k)
    res = run_bass_kernel_spmd(
        nc, in_maps, core_ids=list(range(NCORES)), trace=False
    )
    _cached["last_results"] = res
    return assemble_output([r["out"] for r in res.results])
